# revision 30
# baseline (speedup 1.0000x reference)
"""Trainium2 Bass kernel for nn_DilatedAttentionBlock_attention.

Per-core work (data-parallel over batch, 8 cores):
  x [C=256, L=2048] -> QKV MLPs -> 4-head attention with Lipschitz score
  rescale -> out-proj -> LayerNorm -> ELU + residual -> out [C, L].

Everything stays in channel-major ("transposed") [C, L] layout, which is the
native layout of x_in, so weights act as natural lhsT operands and no input
or output transposes are needed.  Scores are computed directly transposed
(S^T[k, q]) so the softmax exp on the scalar engine doubles as the
PSUM->SBUF copy and the attention matrix never needs transposing for AV.

The schedule is built around the Activation engine being the bottleneck
(the 128 softmax exps of [128, 1024] are ~133us of ACT busy time, more
than the PE's total matmul work).  Projections run K -> Q -> V with
interleaved emission so ct0's alpha (Lipschitz scale) is ready early and
the exp stream starts while V's second linear / transposes still run on
PE/DVE.  Row broadcasts use gpsimd (Pool) partition_broadcast or PE
rank-1 matmuls instead of DRAM DMA bounces.  Tails are emitted
mid-quarter-stream with a tiny ACT footprint; the final tail folds the
LayerNorm affine into PE rank-1 broadcasts (g (x) rstd, g (x) mrs - b)
and uses ACT Square straight out of PSUM for the z^2 moment.

Key algebraic tricks (exact, up to float rounding):
  - row_norm^2[q] = Q[q]^T (K^T K) Q[q] via a tiny 64x64 Gram matrix;
    alpha[q] = 1/sqrt(t[q]) is folded into Q before the score matmul.
  - softmax denominator: V gets a ones-column appended (M=65 AV matmul),
    so row 64 of the AV accumulator is sum_k exp(s).
  - elu: exp(min(u,0)) == min(exp(u), 1) (monotonicity), saving one op.
Matmuls run in float32r / bf16 (full PE column rate) with fp32 PSUM.

SBUF note: tags are heavily cross-phase reused to stay inside the
192KB/partition budget: k_t lives in x's staging slot, v_t in g_q's,
qt in g_k's, y/z in the k/q projection slots, ELU temps in g_v's.
"""

import numpy as np

import concourse.bacc as bacc
import concourse.bass as bass
import concourse.mybir as mybir
import concourse.tile as tile
from concourse.bass_utils import run_bass_kernel_spmd
from concourse.masks import make_identity

B, C, L, H, HD = 8, 256, 2048, 4, 64
P = 128
NCORES = 8
LH = L // 2  # 1024, attention q-half width
FP32 = mybir.dt.float32
FP32R = mybir.dt.float32r
AF = mybir.ActivationFunctionType
OP = mybir.AluOpType

W_NAMES = ["q_w1", "q_w2", "k_w1", "k_w2", "v_w1", "v_w2", "o_w"]
B_NAMES = ["q_b1", "q_b2", "k_b1", "k_b2", "v_b1", "v_b2", "o_b", "ln_g", "ln_b"]

LN_EPS = 1e-5
INV_C = 1.0 / C
BF16 = mybir.dt.bfloat16
SDT = BF16
NMM = 512

PS_TAGS = ["pA0", "pA1", "pB0", "pB1"]


class Ctx:
    """Holds pools + round-robin psum tag allocation."""

    def __init__(self, nc, tc, pools):
        self.nc = nc
        self.tc = tc
        self.staged = False
        (self.consts, self.wpool, self.stage, self.gelu, self.qkv, self.attp,
         self.ps, self.rowp, self.bcp) = pools
        self._ps_i = 0

    def ps_tile(self, shape, name):
        tag = PS_TAGS[self._ps_i % 4]
        self._ps_i += 1
        return self.ps.tile(shape, FP32, tag=tag, name=name, bufs=1)


def _linear_T(cx, w_sb, rhs_tiles, out_tiles, act_fn, bias_sb):
    """out^T[m, l] = act(sum_k w[k, m] * rhs^T[k, l] + bias[m]).

    w_sb [P, 2, C] fp32r; rhs_tiles: 2 tiles [P, L] fp32r (contraction
    k-outer); out_tiles: 2 tiles [P, L].  PSUM in [P, LH] chunks.
    """
    nc = cx.nc
    for m in range(2):
        for lh in range(2):
            acc = cx.ps_tile([P, LH], f"lin_acc_{m}_{lh}")
            for lg in range(2):
                psl = slice(lg * 512, (lg + 1) * 512)
                gsl = slice(lh * LH + lg * 512, lh * LH + (lg + 1) * 512)
                for k in range(2):
                    nc.tensor.matmul(
                        acc[:, psl],
                        w_sb[:, k, m * P : (m + 1) * P],
                        rhs_tiles[k][:, gsl],
                        start=(k == 0),
                        stop=(k == 1),
                    )
            osl = slice(lh * LH, (lh + 1) * LH)
            if act_fn is not None:
                cx.nc.scalar.activation(
                    out_tiles[m][:, osl], acc[:], act_fn,
                    bias=bias_sb[:, m : m + 1],
                )
            else:
                nc.vector.tensor_scalar_add(
                    out_tiles[m][:, osl], acc[:], bias_sb[:, m : m + 1]
                )


def _build_prolog(cx):
    nc = cx.nc

    # ---- biases as [P, 2] columns (swdge path, off the SP queue, issued
    # first so the Pool queue serves them before the weight rounds);
    # ln rows as [1, C] for rank-1 bcast ----
    b_sb = {}
    for name in B_NAMES:
        t = cx.consts.tile([P, 2], FP32, name="b_" + name)
        nc.gpsimd.dma_start(
            t[:], getattr(nc, name + "_t").rearrange("(mo mi) -> mi mo", mi=P)
        )
        b_sb[name] = t
    gb_row_st = cx.consts.tile([1, C], FP32, name="gb_row_st")
    nc.gpsimd.dma_start(gb_row_st[:],
                        nc.ln_g_t.rearrange("(o c) -> o c", o=1))
    g_row = cx.consts.tile([1, C], FP32R, name="g_row")
    b_row = cx.consts.tile([1, C], FP32R, name="b_row")

    # ---- constants (on the otherwise-idle Pool engine) ----
    ident_st = cx.consts.tile([P, P], FP32)
    make_identity(nc, ident_st)
    ident = cx.consts.tile([P, P], FP32R)
    nc.vector.tensor_copy(ident[:], ident_st[:])
    ones_st = cx.consts.tile([P, 32], FP32)
    nc.gpsimd.memset(ones_st[:], 1.0)
    invc_st = cx.consts.tile([P, 1], FP32)
    nc.gpsimd.memset(invc_st[:], INV_C)
    invc_128 = cx.consts.tile([P, 1], FP32R)
    nc.vector.tensor_copy(invc_128[:], invc_st[:])
    eps_sb = cx.consts.tile([1, 1], FP32)
    nc.gpsimd.memset(eps_sb[:], LN_EPS)
    mones_st = cx.rowp.tile([1, LH], FP32, tag="lnt0", name="mones_st",
                            bufs=1)
    nc.gpsimd.memset(mones_st[:], -1.0)
    mones_row = cx.consts.tile([1, LH], FP32R)
    nc.vector.tensor_copy(mones_row[:], mones_st[:])
    # sel2 [P, 2]: column h selects head-h's 64 partitions (for t row sums)
    sel_st = cx.consts.tile([P, 2], FP32)
    nc.gpsimd.memset(sel_st[:], 0.0)
    nc.gpsimd.memset(sel_st[0:64, 0:1], 1.0)
    nc.gpsimd.memset(sel_st[64:128, 1:2], 1.0)
    sel2 = cx.consts.tile([P, 2], FP32R)
    nc.vector.tensor_copy(sel2[:], sel_st[:])
    # selbc [2, P]: row r broadcasts to partition group r (alpha broadcast).
    # Partition-1-only writes are illegal for memset, so build it as the
    # PE transpose of sel2.
    selbc = cx.consts.tile([2, P], FP32R)
    trsel = cx.ps_tile([2, P], "trsel")
    nc.tensor.transpose(trsel.bitcast(FP32R)[:], sel2[:], ident[:])
    nc.vector.tensor_copy(selbc[:], trsel.bitcast(FP32R)[:])
    gz_st = cx.consts.tile([P, P], FP32)
    nc.gpsimd.memset(gz_st[:], 0.0)
    onesr_st = cx.consts.tile([1, 64], FP32)
    nc.gpsimd.memset(onesr_st[:], 1.0)
    ones_64row = cx.consts.tile([1, 64], FP32R)
    nc.vector.tensor_copy(ones_64row[:], onesr_st[:])
    nc.vector.tensor_copy(g_row[:], gb_row_st[:])
    nc.gpsimd.dma_start(gb_row_st[:],
                        nc.ln_b_t.rearrange("(o c) -> o c", o=1))
    nc.vector.tensor_copy(b_row[:], gb_row_st[:])

    # ---- weights: staged DMA + DVE rounding copy, once per kernel ----
    def load_w(name, tag):
        st = cx.wpool.tile([P, 2, C], FP32, tag="w_stage", bufs=1,
                           name=f"wst_{name}")
        nc.sync.dma_start(
            st[:],
            getattr(nc, name + "_t").rearrange("(ko ki) m -> ki ko m", ki=P),
        )
        wr = cx.wpool.tile([P, 2, C], FP32R, tag=tag, name=f"w_{name}", bufs=1)
        nc.vector.tensor_copy(wr[:], st[:])
        return wr

    w_k1 = load_w("k_w1", "wA")
    w_q1 = load_w("q_w1", "wB")
    w_k2 = load_w("k_w2", "wC")
    w_v1 = load_w("v_w1", "wD")
    w_q2 = load_w("q_w2", "wE")
    w_v2 = load_w("v_w2", "wF")
    ow_sb = load_w("o_w", "w_ow")
    return dict(b_sb=b_sb, g_row=g_row, b_row=b_row, ident=ident,
                ones_st=ones_st, invc_128=invc_128, eps_sb=eps_sb,
                mones_row=mones_row, sel2=sel2, selbc=selbc, gz_st=gz_st,
                ones_64row=ones_64row, w_k1=w_k1, w_q1=w_q1, w_k2=w_k2,
                w_v1=w_v1, w_q2=w_q2, w_v2=w_v2, ow_sb=ow_sb)


def _build_body(cx, env, phases=4):
    nc = cx.nc
    x_in, out = nc.x_in_t, nc.out_t
    (b_sb, g_row, b_row, ident, ones_st, invc_128, eps_sb, mones_row, sel2,
     selbc, gz_st, ones_64row, w_k1, w_q1, w_k2, w_v1, w_q2, w_v2, ow_sb) = (
        env[k] for k in (
            "b_sb", "g_row", "b_row", "ident", "ones_st", "invc_128",
            "eps_sb", "mones_row", "sel2", "selbc", "gz_st", "ones_64row",
            "w_k1", "w_q1", "w_k2", "w_v1", "w_q2", "w_v2", "ow_sb"))

    def _dump(tiles):
        for m, t in enumerate(tiles):
            v = t.bitcast(FP32)
            nc.sync.dma_start(out[m * P : (m + 1) * P, 0 : v.shape[-1]], v[:])

    # ---- x load (l-chunked); prefetchable in repeat mode once the
    # previous iteration's gram has released the staging slot ----
    x_re = x_in.rearrange("(ko ki) l -> ki ko l", ki=P)
    xst = cx.stage.tile([P, 2, L], FP32, tag="x_st")
    xr = cx.stage.tile([P, 2, L], FP32R, tag="xr")
    NXC = 4
    XC = L // NXC
    for c in range(NXC):
        xsl = slice(c * XC, (c + 1) * XC)
        nc.sync.dma_start(xst[:, :, xsl], x_re[:, :, xsl])
        nc.vector.tensor_copy(xr[:, :, xsl], xst[:, :, xsl])

    # ---- MLPs, K -> Q -> (V.w1+gelu), interleaved on the PE queue ----
    g_k = [cx.gelu.tile([P, L], FP32R, tag=f"gk{m}", name=f"g_k{m}", bufs=1)
           for m in range(2)]
    g_q = [cx.gelu.tile([P, L], FP32R, tag=f"gq{m}", name=f"g_q{m}", bufs=1)
           for m in range(2)]
    g_v = [cx.gelu.tile([P, L], FP32R, tag=f"gv{m}", name=f"g_v{m}", bufs=1)
           for m in range(2)]
    k_sb = [cx.qkv.tile([P, L], FP32R, tag=f"k{m}", name=f"k_sb{m}", bufs=1)
            for m in range(2)]
    q_sb = [cx.qkv.tile([P, L], FP32R, tag=f"q{m}", name=f"q_sb{m}", bufs=1)
            for m in range(2)]
    v_sb = [cx.qkv.tile([P, L], FP32R, tag=f"v{m}", name=f"v_sb{m}", bufs=1)
            for m in range(2)]

    _linear_T(cx, w_k1, [xr[:, 0], xr[:, 1]], g_k, AF.Gelu, b_sb["k_b1"])
    _linear_T(cx, w_q1, [xr[:, 0], xr[:, 1]], g_q, AF.Gelu, b_sb["q_b1"])
    _linear_T(cx, w_k2, g_k, k_sb, None, b_sb["k_b2"])

    # kb: bf16 K for the score matmuls (per-ct DVE copies, emitted where
    # the DVE queue has slack)
    kb = cx.qkv.tile([P, 2, L], SDT, tag="kb", name="kb", bufs=1)

    def kb_copy(ct):
        nc.vector.tensor_copy(kb[:, ct, :], k_sb[ct][:])

    # ---- K transposes + Gram (PE); k_t reuses x's staging slot and is
    # single-buffered (gram ct0 completes before ktrans ct1 overwrites).
    # v1's matmuls are emitted between ktrans0 and gram0 so the PE isn't
    # idle while the DVE runs the k_t copies. ----
    g_pair = [None, None]
    _kt_state = {}

    def ktrans(ct):
        kt_tile = cx.stage.tile([P, 16, P], FP32R, tag="x_st",
                                name=f"k_t{ct}", bufs=1)
        _kt_state[ct] = kt_tile
        for lt0 in range(0, 16, 4):
            trk = cx.ps_tile([P, 512], f"trk_{ct}_{lt0}")
            for j in range(4):
                nc.tensor.transpose(
                    trk.bitcast(FP32R)[:, j * P : (j + 1) * P],
                    k_sb[ct][:, (lt0 + j) * P : (lt0 + j + 1) * P],
                    ident[:],
                )
            nc.vector.tensor_copy(
                kt_tile[:, lt0 : lt0 + 4, :],
                trk.bitcast(FP32R).rearrange("p (l c) -> p l c", l=4),
            )

    def gram(ct):
        kt_tile = _kt_state[ct]
        g_ps = cx.ps_tile([P, P], f"g_ps{ct}")
        for kt in range(16):
            nc.tensor.matmul(
                g_ps[:], kt_tile[:, kt, :], kt_tile[:, kt, :],
                start=(kt == 0), stop=(kt == 15),
            )
        gp = cx.rowp.tile([P, P], FP32R, tag=f"gram{ct}", name=f"g_pair{ct}",
                          bufs=1)
        nc.vector.tensor_copy(gp[:], gz_st[:])
        for ho in range(2):
            hsl = slice(64 * ho, 64 * ho + 64)
            nc.vector.tensor_copy(gp[hsl, hsl], g_ps.bitcast(FP32R)[hsl, hsl])
        g_pair[ct] = gp

    kb_copy(0)
    _linear_T(cx, w_v1, [xr[:, 0], xr[:, 1]], g_v, AF.Gelu, b_sb["v_b1"])
    ktrans(0)
    _linear_T(cx, w_q2, g_q, q_sb, None, b_sb["q_b2"])
    gram(0)

    # ---- alpha(ct) -> qt(ct): GQ, t, ln, exp, PE rank-1 broadcast ----
    qt_sb = [cx.gelu.tile([P, L], SDT, tag=f"gk{ct}", name=f"qt{ct}", bufs=1)
             for ct in range(2)]

    _alpha_state = {}

    def alpha_qt(ct, upto=None, frm=None):
        # stage "t": GQ + t matmuls + the ACT Ln rows; stage "rows":
        # in-place exp + PE broadcast + qt multiply.
        if frm is None:
            lnt = [cx.rowp.tile([2, LH], FP32R, tag=f"lnt{lh}",
                                name=f"lnt{ct}{lh}", bufs=1)
                   for lh in range(2)]
            _alpha_state[ct] = lnt
            for lh in range(2):
                lsl = slice(lh * LH, (lh + 1) * LH)
                qgq = cx.rowp.tile([P, LH], FP32R, tag="qgq",
                                   name=f"qgq{ct}{lh}", bufs=1)
                gq_ps = cx.ps_tile([P, LH], f"gq_ps{ct}{lh}")
                for lg in range(2):
                    psl = slice(lg * 512, (lg + 1) * 512)
                    gsl = slice(lh * LH + lg * 512, lh * LH + (lg + 1) * 512)
                    nc.tensor.matmul(gq_ps[:, psl], g_pair[ct][:],
                                     q_sb[ct][:, gsl], start=True, stop=True)
                nc.vector.tensor_tensor(out=qgq[:], in0=q_sb[ct][:, lsl],
                                        in1=gq_ps[:], op=OP.mult)
                t_ps = cx.ps_tile([2, LH], f"t_ps{ct}{lh}")
                for lg in range(2):
                    psl = slice(lg * 512, (lg + 1) * 512)
                    nc.tensor.matmul(t_ps[:, psl], sel2[:], qgq[:, psl],
                                     start=True, stop=True)
                nc.scalar.activation(lnt[lh][:], t_ps[:], AF.Ln)
            if upto == "t":
                return
        lnt = _alpha_state[ct]
        # in-place exp + PE broadcast of row h to partition group h
        for lh in range(2):
            lsl = slice(lh * LH, (lh + 1) * LH)
            nc.scalar.activation(lnt[lh][:], lnt[lh][:], AF.Exp, scale=-0.5)
            abc_ps = cx.ps_tile([P, LH], f"abc{ct}{lh}")
            for lg in range(2):
                psl = slice(lg * 512, (lg + 1) * 512)
                nc.tensor.matmul(
                    abc_ps[:, psl], selbc[:],
                    lnt[lh][:, psl],
                    start=True, stop=True)
            nc.vector.tensor_tensor(out=qt_sb[ct][:, lsl],
                                    in0=q_sb[ct][:, lsl], in1=abc_ps[:],
                                    op=OP.mult)

    alpha_qt(0, upto="t")

    _linear_T(cx, w_v2, g_v, v_sb, None, b_sb["v_b2"])

    alpha_qt(0, frm="rows")
    kb_copy(1)

    # ---- V transposes (ones-augmented); v_t reuses g_q's slots ----
    v_t = [None, None]

    def vtrans(ct):
        vt_tile = cx.gelu.tile([P, 16, 130], SDT, tag=f"gq{ct}",
                               name=f"v_t{ct}", bufs=1)
        nc.vector.tensor_copy(
            vt_tile.rearrange("p l (h c) -> p l h c", h=2)[:, :, :, 64:65],
            ones_st.rearrange("p (l h c) -> p l h c", l=16, h=2),
        )
        for lt0 in range(0, 16, 4):
            trv = cx.ps_tile([P, 512], f"trv_{ct}_{lt0}")
            for j in range(4):
                nc.tensor.transpose(
                    trv.bitcast(FP32R)[:, j * P : (j + 1) * P],
                    v_sb[ct][:, (lt0 + j) * P : (lt0 + j + 1) * P],
                    ident[:],
                )
            nc.vector.tensor_copy(
                vt_tile[:, lt0 : lt0 + 4, :]
                .rearrange("p l (h c) -> p l h c", h=2)[:, :, :, 0:64],
                trv.bitcast(FP32R).rearrange("p (l h c) -> p l h c", l=4, h=2),
            )
        v_t[ct] = vt_tile

    vtrans(0)
    ktrans(1)

    if phases == 1:
        _dump(q_sb)
        return
    if phases == 2:
        gram(1)
        alpha_qt(1)
        _dump(qt_sb)
        return

    # ---- attention + per-half tails (y/z reuse the k/q slots) ----
    y_sb = [cx.qkv.tile([P, L], FP32R, tag=f"k{ct}", name=f"y{ct}", bufs=1)
            for ct in range(2)]
    z_sb = [cx.qkv.tile([P, L], FP32R, tag=f"q{m}", name=f"z{m}", bufs=1)
            for m in range(2)]

    def quarter(qh, ct, ho, pe_div=False):
        """One head, one q-half.  Score accumulator double-buffered on kt
        parity; AV lags one kt so it is never exp-gated at the head of the
        in-order PE queue.  direct=True multiplies y straight out of the
        AV PSUM (no yc staging copy) for the last quarters of the kernel,
        where nothing competes for the pB tags."""
        q0 = qh * LH
        hslice = slice(q0, q0 + LH)
        hsl = slice(64 * ho, 64 * ho + 64)
        b_ps = cx.ps.tile([65, LH], FP32, tag=PS_TAGS[2 + ho],
                          name=f"av{ct}{qh}{ho}", bufs=1)

        def s_mm(kt):
            a = cx.ps.tile([P, LH], FP32, tag=PS_TAGS[kt % 2],
                           name=f"s{ct}{qh}{kt}{ho}", bufs=1)
            for lg in range(LH // NMM):
                psl = slice(lg * NMM, (lg + 1) * NMM)
                nc.tensor.matmul(
                    a[:, psl],
                    kb[hsl, ct, kt * P : (kt + 1) * P],
                    qt_sb[ct][hsl, q0 + lg * NMM : q0 + (lg + 1) * NMM],
                    start=True, stop=True,
                )
            return a

        def av_mm(kt, attn):
            for lg in range(LH // NMM):
                psl = slice(lg * NMM, (lg + 1) * NMM)
                nc.tensor.matmul(
                    b_ps[:, psl],
                    v_t[ct][:, kt, 65 * ho : 65 * ho + 65],
                    attn[:, psl],
                    start=(kt == 0), stop=(kt == 15),
                )

        a_cur = s_mm(0)
        attn_prev = None
        for kt in range(16):
            attn = cx.attp.tile([P, LH], SDT, tag=f"attn{kt % 2}",
                                name=f"at{ct}{qh}{kt}{ho}", bufs=2)
            nc.scalar.activation(attn[:], a_cur[:], AF.Exp)
            if kt < 15:
                a_cur = s_mm(kt + 1)
            if attn_prev is not None:
                av_mm(kt - 1, attn_prev)
            attn_prev = attn
        av_mm(15, attn_prev)

        # drain: yc out of PSUM promptly; 1/d broadcast on the Pool
        # engine (or, for the kernel's last quarter, via a PE rank-1
        # matmul into the now-free score tags -- shorter latency)
        invd = cx.rowp.tile([1, LH], FP32R, tag="rowA",
                            name=f"invd{ct}{qh}{ho}", bufs=1)
        with nc.allow_low_precision(reason="fp32r 1/d row: ~tf32 precision "
                                    "is far inside the softmax tolerance"):
            nc.vector.reciprocal(invd[:], b_ps[64:65, :])
        yc = cx.rowp.tile([64, LH], FP32,
                          tag=("qgq" if ho == 0 else "z2b"),
                          name=f"yc{qh}{ct}{ho}", bufs=1)
        nc.vector.tensor_copy(yc[:], b_ps[0:64, :])
        if pe_div:
            dbc_ps = cx.ps.tile([64, LH], FP32, tag=PS_TAGS[0],
                                name=f"dbcp{ct}{qh}{ho}", bufs=1)
            for lg in range(2):
                psl = slice(lg * 512, (lg + 1) * 512)
                nc.tensor.matmul(dbc_ps[:, psl], ones_64row[:],
                                 invd[:, psl], start=True, stop=True)
            nc.vector.tensor_tensor(
                out=y_sb[ct][hsl, hslice], in0=yc[:], in1=dbc_ps[:],
                op=OP.mult,
            )
        else:
            dbc = cx.bcp.tile([64, LH], FP32, tag="bc",
                              name=f"dbc{ct}{qh}{ho}", bufs=2)
            nc.gpsimd.partition_broadcast(dbc[:], invd[:].bitcast(FP32))
            nc.vector.tensor_tensor(
                out=y_sb[ct][hsl, hslice], in0=yc[:], in1=dbc[:], op=OP.mult,
            )

    def tail_zacc(qh, final):
        """out-proj + PSUM drains; no ACT ops (except the final z^2
        Squares), so it can be emitted well before the ACT stream reaches
        the matching tail_act point."""
        q0 = qh * LH
        hslice = slice(q0, q0 + LH)
        zps = []
        for m in range(2):
            acc = cx.ps.tile([P, LH], FP32, tag=PS_TAGS[2 + m],
                             name=f"zacc{qh}{m}", bufs=1)
            for lg in range(2):
                psl = slice(lg * 512, (lg + 1) * 512)
                gsl = slice(q0 + lg * 512, q0 + (lg + 1) * 512)
                for ct in range(2):
                    nc.tensor.matmul(
                        acc[:, psl],
                        ow_sb[:, ct, m * P : (m + 1) * P],
                        y_sb[ct][:, gsl],
                        start=(ct == 0), stop=(ct == 1),
                    )
            zps.append(acc)
        z2h = [cx.rowp.tile([P, LH], FP32R,
                            tag=("qgq" if m == 0 else "z2b"),
                            name=f"z2_{qh}{m}", bufs=1) for m in range(2)]
        for m in range(2):
            nc.vector.tensor_scalar_add(z_sb[m][:, hslice], zps[m][:],
                                        b_sb["o_b"][:, m : m + 1])
        if final:
            # ACT is idle in the suffix: square straight out of PSUM
            for m in range(2):
                nc.scalar.activation(z2h[m][:], zps[m][:], AF.Square,
                                     bias=b_sb["o_b"][:, m : m + 1])
        else:
            for m in range(2):
                nc.vector.tensor_tensor(out=z2h[m][:], in0=z_sb[m][:, hslice],
                                        in1=z_sb[m][:, hslice], op=OP.mult)
        return z2h

    def tail_stats(qh, final, z2h):
        q0 = qh * LH
        s1_ps = cx.ps.tile([1, LH], FP32, tag=PS_TAGS[2], name=f"s1_{qh}",
                           bufs=1)
        s2_ps = cx.ps.tile([1, LH], FP32, tag=PS_TAGS[3], name=f"s2_{qh}",
                           bufs=1)
        for lg in range(2):
            psl = slice(lg * 512, (lg + 1) * 512)
            gsl = slice(q0 + lg * 512, q0 + (lg + 1) * 512)
            for m in range(2):
                nc.tensor.matmul(s1_ps[:, psl], invc_128[:], z_sb[m][:, gsl],
                                 start=(m == 0), stop=(m == 1))
            for m in range(2):
                nc.tensor.matmul(s2_ps[:, psl], invc_128[:], z2h[m][:, psl],
                                 start=(m == 0), stop=(m == 1))
        mu = cx.rowp.tile([1, LH], FP32, tag="rowB", name=f"mu{qh}", bufs=1)
        nc.vector.tensor_copy(mu[:], s1_ps[:])
        var = cx.rowp.tile([1, LH], FP32, tag="lnt0", name=f"var{qh}", bufs=1)
        if final:
            # ACT is otherwise idle here; Square reads straight from PSUM
            # in parallel with the DVE mu copy
            nc.scalar.activation(var[:], s1_ps[:], AF.Square)
        else:
            nc.vector.tensor_tensor(out=var[:], in0=mu[:], in1=mu[:],
                                    op=OP.mult)
        var2 = cx.rowp.tile([1, LH], FP32, tag="z2b", name=f"var2{qh}",
                            bufs=1)
        nc.vector.scalar_tensor_tensor(out=var2[:], in0=s2_ps[:], scalar=0.0,
                                       in1=var[:], op0=OP.add,
                                       op1=OP.subtract)
        return mu, var2

    def tail_act(qh, final, mu, var2):
        """ln/exp rows, broadcasts, affine, elu, out DMA."""
        q0 = qh * LH
        hslice = slice(q0, q0 + LH)
        nc.scalar.activation(var2[:], var2[:], AF.Ln, bias=eps_sb[:])
        rstd = cx.rowp.tile([1, LH], FP32R, tag="rowC", name=f"rstd{qh}",
                            bufs=1)
        nc.scalar.activation(rstd[:], var2[:], AF.Exp, scale=-0.5)
        if final:
            # rank-1 PE broadcasts: A = g (x) rstd first (only needs rstd),
            # then mrs on DVE in parallel, then Cm = g (x) mrs - b (x) 1.
            a_ps, c_ps = [], []
            for m in range(2):
                a_ps.append(cx.ps.tile([P, LH], FP32, tag=PS_TAGS[m],
                                       name=f"Abc{qh}{m}", bufs=1))
                for lg in range(2):
                    psl = slice(lg * 512, (lg + 1) * 512)
                    nc.tensor.matmul(a_ps[m][:, psl],
                                     g_row[:, m * P : (m + 1) * P],
                                     rstd[:, psl], start=True, stop=True)
        mrs = cx.rowp.tile([1, LH], FP32R, tag="rowA", name=f"mrs{qh}",
                           bufs=1)
        nc.vector.tensor_tensor(out=mrs[:], in0=mu[:], in1=rstd[:],
                                op=OP.mult)

        if final:
            for m in range(2):
                c_ps.append(cx.ps.tile([P, LH], FP32, tag=PS_TAGS[2 + m],
                                       name=f"Cbc{qh}{m}", bufs=1))
                for lg in range(2):
                    psl = slice(lg * 512, (lg + 1) * 512)
                    nc.tensor.matmul(c_ps[m][:, psl],
                                     g_row[:, m * P : (m + 1) * P],
                                     mrs[:, psl], start=True, stop=False)
                    nc.tensor.matmul(c_ps[m][:, psl],
                                     b_row[:, m * P : (m + 1) * P],
                                     mones_row[:, psl], start=False, stop=True)
            es = [cx.gelu.tile([P, LH], FP32, tag=f"gv{m}",
                               name=f"e{qh}{m}", bufs=1) for m in range(2)]
            for lg in range(2):
                psl = slice(lg * 512, (lg + 1) * 512)
                osl = slice(q0 + lg * 512, q0 + (lg + 1) * 512)
                for m in range(2):
                    u = y_sb[m][:, osl]
                    e = es[m][:, psl]
                    nc.vector.tensor_tensor(out=u[:], in0=z_sb[m][:, osl],
                                            in1=a_ps[m][:, psl], op=OP.mult)
                    nc.vector.tensor_tensor(out=u[:], in0=u[:],
                                            in1=c_ps[m][:, psl],
                                            op=OP.subtract)
                    nc.scalar.activation(e[:], u[:], AF.Exp)
                    nc.vector.tensor_scalar_min(e[:], e[:], 1.0)
                    nc.vector.scalar_tensor_tensor(out=u[:], in0=u[:],
                                                   scalar=0.0, in1=e[:],
                                                   op0=OP.max, op1=OP.add)
                    nc.vector.scalar_tensor_tensor(out=u[:], in0=u[:],
                                                   scalar=-1.0,
                                                   in1=xr[:, m, osl],
                                                   op0=OP.add, op1=OP.add)
                    nc.gpsimd.dma_start(out[m * P : (m + 1) * P, osl],
                                        u.bitcast(FP32)[:])
                    warm = cx.ps.tile([P, 512], FP32, tag=PS_TAGS[m],
                                      name=f"warm{qh}{m}{lg}", bufs=1)
                    nc.tensor.matmul(warm[:], ident[:], u[:, 0:512],
                                     start=True, stop=True)
        else:
            rstdb = cx.bcp.tile([P, LH], FP32, tag="bc", name=f"rstdb{qh}",
                                bufs=2)
            nc.gpsimd.partition_broadcast(rstdb[:], rstd[:].bitcast(FP32))
            mrsb = cx.bcp.tile([P, LH], FP32, tag="bc", name=f"mrsb{qh}",
                               bufs=2)
            nc.gpsimd.partition_broadcast(mrsb[:], mrs[:].bitcast(FP32))
            for m in range(2):
                u = y_sb[m][:, hslice]
                nc.vector.tensor_tensor(out=u[:], in0=z_sb[m][:, hslice],
                                        in1=rstdb[:], op=OP.mult)
                nc.vector.tensor_tensor(out=u[:], in0=u[:], in1=mrsb[:],
                                        op=OP.subtract)
                nc.vector.tensor_scalar(
                    out=u[:], in0=u[:],
                    scalar1=b_sb["ln_g"][:, m : m + 1],
                    scalar2=b_sb["ln_b"][:, m : m + 1],
                    op0=OP.mult, op1=OP.add,
                )
            for m in range(2):
                u = y_sb[m][:, hslice]
                e = cx.gelu.tile([P, LH], FP32, tag=f"gv{m}",
                                 name=f"e{qh}{m}", bufs=1)
                nc.scalar.activation(e[:], u[:], AF.Exp)
                nc.vector.tensor_scalar_min(e[:], e[:], 1.0)
                nc.vector.scalar_tensor_tensor(out=u[:], in0=u[:], scalar=0.0,
                                               in1=e[:], op0=OP.max,
                                               op1=OP.add)
                nc.vector.scalar_tensor_tensor(out=u[:], in0=u[:], scalar=-1.0,
                                               in1=xr[:, m, hslice],
                                               op0=OP.add, op1=OP.add)
                nc.gpsimd.dma_start(out[m * P : (m + 1) * P, hslice],
                                    u.bitcast(FP32)[:])

    if phases == 3:
        gram(1)
        alpha_qt(1)
        vtrans(1)
        for qh in range(2):
            for ct in range(2):
                for ho in range(2):
                    quarter(qh, ct, ho)
        _dump(y_sb)
        return

    quarter(0, 0, 0)
    gram(1)
    alpha_qt(1)
    quarter(0, 0, 1)
    vtrans(1)
    cx.tc.stage_boundary() if cx.staged else None
    quarter(0, 1, 0)
    quarter(0, 1, 1)
    z2h0 = tail_zacc(0, final=False)
    cx.tc.stage_boundary() if cx.staged else None
    quarter(1, 0, 0)
    t0 = tail_stats(0, False, z2h0)
    quarter(1, 0, 1)
    tail_act(0, False, *t0)
    cx.tc.stage_boundary() if cx.staged else None
    quarter(1, 1, 0)
    quarter(1, 1, 1, pe_div=True)
    z2h1 = tail_zacc(1, final=True)
    t1 = tail_stats(1, True, z2h1)
    tail_act(1, True, *t1)


def _steer_act_tables():
    """The act-table-load pass picks the first set containing each
    function, which thrashes natural_log <-> exp_and_others when a kernel
    uses both Ln and Exp.  Empty out the single-function sets so both
    resolve to natural_log_exp_and_others (ids keep their positions)."""
    import concourse.hw_specs as hw_specs
    if getattr(hw_specs, "_act_tables_steered", False):
        return
    orig = hw_specs.get_activation_tables

    def patched(arch):
        t = dict(orig(arch))
        for k in ("natural_log", "exp_and_others", "exp_and_friends"):
            if k in t:
                t[k] = set()
        return t

    hw_specs.get_activation_tables = patched
    bacc.get_activation_tables = patched
    hw_specs._act_tables_steered = True


def build_nc(repeat: int = 1, phases: int = 4):
    _steer_act_tables()
    nc = bacc.Bacc("TRN2", target_bir_lowering=False)
    nc.x_in_t = nc.dram_tensor("x_in", [C, L], FP32, kind="ExternalInput")
    for name in W_NAMES:
        setattr(nc, name + "_t",
                nc.dram_tensor(name, [C, C], FP32, kind="ExternalInput"))
    for name in B_NAMES:
        setattr(nc, name + "_t",
                nc.dram_tensor(name, [C], FP32, kind="ExternalInput"))
    nc.out_t = nc.dram_tensor("out", [C, L], FP32, kind="ExternalOutput")

    with tile.TileContext(nc) as tc:
        with (
            tc.tile_pool(name="consts", bufs=1) as consts,
            tc.tile_pool(name="wpool", bufs=1) as wpool,
            tc.tile_pool(name="stage", bufs=1) as stage,
            tc.tile_pool(name="gelu", bufs=1) as gelu,
            tc.tile_pool(name="qkv", bufs=1) as qkv,
            tc.tile_pool(name="attp", bufs=1) as attp,
            tc.tile_pool(name="ps", bufs=1, space="PSUM") as ps,
            tc.tile_pool(name="rowp", bufs=1) as rowp,
            tc.tile_pool(name="bcp", bufs=1) as bcp,
        ):
            pools = (consts, wpool, stage, gelu, qkv, attp, ps, rowp, bcp)
            cx = Ctx(nc, tc, pools)
            env = _build_prolog(cx)
            if repeat == 1:
                _build_body(cx, env, phases)
            else:
                with tc.For_i(0, repeat, 1,
                              hint_engines=(mybir.EngineType.PE,
                                            mybir.EngineType.Activation,
                                            mybir.EngineType.DVE,
                                            mybir.EngineType.Pool)):
                    _build_body(cx, env, phases)
    nc.finalize()
    return nc


_NC_CACHE = {}


def _get_nc(repeat=1, phases=4):
    key = (repeat, phases)
    if key not in _NC_CACHE:
        _NC_CACHE[key] = build_nc(repeat, phases)
    return _NC_CACHE[key]


def kernel(**inputs: np.ndarray) -> np.ndarray:
    nc = _get_nc()
    x_in = np.ascontiguousarray(inputs["x_in"], dtype=np.float32)
    shared = {}
    for name in W_NAMES + B_NAMES:
        shared[name] = np.ascontiguousarray(inputs[name], dtype=np.float32)
    in_maps = [dict(shared, x_in=x_in[b]) for b in range(NCORES)]
    res = run_bass_kernel_spmd(nc, in_maps, core_ids=list(range(NCORES)))
    return np.stack([res.results[b]["out"] for b in range(NCORES)], axis=0)
